# revision 44
# baseline (speedup 1.0000x reference)
"""Trainium2 Bass kernel for EnhancedAttention (B=2, T=2048, D=1024, H=16, DH=64).

Sharding: 8 cores = 2 batches x 4 head-groups (4 heads each). No collectives;
each core computes a partial out-projection and the host sums the 4 partials
per batch.

v3: flat zippered attention pipeline across (chunk, head) units so the PE
stream stays dense (clock ramped):
  - unit (qc,h)'s S-matmul groups interleave with unit-prev's AV groups,
    including across chunk boundaries
  - proj(c+1)/outproj(c') pieces fill remaining slack
  - S/AV emitted in groups of 2 kb-pairs to amortize PE row-config switches
  - denominator reciprocal batched [4,512]; broadcast via tiny matmuls
  - ot partition layout par-swapped (host reorders Wout rows) so the last
    head's normalize writes SBUF directly
"""
import os
import sys

for _p in ("/opt/trn_rl_repo", "/root/.axon_site/_ro/trn_rl_repo"):
    if os.path.isdir(_p) and _p not in sys.path:
        sys.path.append(_p)

import ml_dtypes
import numpy as np

import concourse.bass as bass  # noqa: F401
import concourse.tile as tile
from concourse import bacc, mybir
from concourse.bass_utils import run_bass_kernel_spmd

B, T, D = 2, 2048, 1024
H, DH = 16, 64
HPC = 4  # heads per core
NCORES = 8
ROPE_THETA = 10000.0

F32 = mybir.dt.float32
BF16 = mybir.dt.bfloat16
FP8 = mybir.dt.float8e4

TCH = 512  # t-chunk (q-chunk) size
TC = T // TCH  # 4
DC = D // 128  # 8 contraction chunks
NKT = T // 128  # 16 k-tiles

CHUNK_ORDER = [1, 2, 3, 0]


def _rope_tables():
    inv = 1.0 / (ROPE_THETA ** (np.arange(0, DH, 2, dtype=np.float64) / DH))
    f = np.arange(T, dtype=np.float64)[:, None] * inv[None, :]  # [T, 32]
    import ml_dtypes as _md
    cos = np.cos(f).T.astype(_md.bfloat16)  # [32, T]
    sin = np.sin(f).T.astype(_md.bfloat16)
    cs1 = np.ascontiguousarray(np.tile(cos, (4, 1)))  # [128, T]
    cs2 = np.ascontiguousarray(np.concatenate([sin, -sin, sin, -sin], axis=0))
    return cs1, cs2


def _build():
    nc = bacc.Bacc("TRN2", target_bir_lowering=False, debug=False, num_devices=NCORES)
    xT_d = nc.dram_tensor("xT", [D, T], BF16, kind="ExternalInput")
    wq_d = nc.dram_tensor("wq", [D, HPC * DH], BF16, kind="ExternalInput")
    wk_d = nc.dram_tensor("wk", [D, HPC * DH], BF16, kind="ExternalInput")
    wv_d = nc.dram_tensor("wv", [D, HPC * DH], BF16, kind="ExternalInput")
    wo_d = nc.dram_tensor("wo", [HPC * DH, D], BF16, kind="ExternalInput")
    y_d = nc.dram_tensor("y", [T, D], F32, kind="ExternalOutput")

    cs1_np, cs2_np = _rope_tables()
    # unique rows only: cs1 is [32,T] tiled 4x, cs2 is [64,T] tiled 2x
    cs1_d = nc.inline_tensor(np.ascontiguousarray(cs1_np[:32]), "cs1")
    cs2_d = nc.inline_tensor(np.ascontiguousarray(cs2_np[:64]), "cs2")

    EXP = mybir.ActivationFunctionType.Exp
    COPYF = mybir.ActivationFunctionType.Copy

    import contextlib
    with tile.TileContext(nc) as tc:
        with (
            contextlib.ExitStack() as _ctx,
            tc.tile_pool(name="sb", bufs=1) as sb,
            tc.tile_pool(name="xtp", bufs=2) as xtp,
            tc.tile_pool(name="ropep", bufs=2) as ropep,
            tc.tile_pool(name="ptp", bufs=18) as ptp,
            tc.tile_pool(name="orawp", bufs=6) as orawp,
            tc.tile_pool(name="miscp", bufs=2) as miscp,
            tc.tile_pool(name="ysbp", bufs=2) as ysbp,
        ):
            wq = sb.tile([128, DC, HPC * DH], BF16)
            wk = sb.tile([128, DC, HPC * DH], BF16)
            wv = sb.tile([128, DC, HPC * DH], BF16)
            wo = sb.tile([128, 2, D], BF16)
            cs1 = sb.tile([128, T], BF16)
            cs2 = sb.tile([128, T], BF16)
            # folded fp8 layout for DoubleRow score matmuls: tile hp holds
            # heads 2hp (partitions 0-31) and 2hp+1 (32-63); free dim1 holds
            # the dh pair slot (dh = slot*32 + partition). Two tiles per
            # tensor because matmul operand base partitions max out at 64.
            qt8 = [sb.tile([64, 2, T], FP8, name=f"qt8_{hp}") for hp in range(2)]
            kt8 = [sb.tile([64, 2, T], FP8, name=f"kt8_{hp}") for hp in range(2)]
            vaug = sb.tile([128, NKT, HPC, DH + 1], BF16)
            ot = [sb.tile([128, T], BF16, tag=f"ot{p}", name=f"ot{p}") for p in range(2)]
            # selector constants for matmul-based denominator broadcast:
            # bcps[hp] partitions 0-63 get head 2hp+1's recip (par-swapped ot
            # layout), partitions 64-127 get head 2hp's
            sel = [sb.tile([2, 128], F32, tag=f"sel{hp}", name=f"sel{hp}") for hp in range(2)]

            def load_sel():
                for hp in range(2):
                    sel_np = np.zeros((2, 128), dtype=np.float32)
                    sel_np[1, 0:64] = 1.0
                    sel_np[0, 64:128] = 1.0
                    sel_d = nc.inline_tensor(sel_np, f"selc{hp}")
                    nc.scalar.dma_start(sel[hp][:], sel_d.ap())

            # preloads in need-order: wq first (piece 1), cs (rope, ~12us),
            # wk/wv on the gpsimd ring; wo/sel deferred to unit-0 fillers
            nc.scalar.dma_start(wq[:], wq_d.ap().rearrange("(c p) n -> p c n", p=128))
            nc.scalar.dma_start(cs1[0:32, :], cs1_d.ap())
            nc.scalar.dma_start(cs2[0:64, :], cs2_d.ap())
            for rep in range(1, 4):
                nc.scalar.dma_start(cs1[rep * 32 : (rep + 1) * 32, :], cs1[0:32, :])
            nc.scalar.dma_start(cs2[64:128, :], cs2[0:64, :])
            nc.gpsimd.dma_start(wk[:], wk_d.ap().rearrange("(c p) n -> p c n", p=128))

            def load_wv():
                # sits behind piece-2's k-swap DMAs on the gpsimd queue, so
                # it doesn't steal startup bandwidth from wq/xt0/wk
                nc.gpsimd.dma_start(wv[:], wv_d.ap().rearrange("(c p) n -> p c n", p=128))

            nc.vector.memset(vaug[:, :, :, DH : DH + 1], 1.0)

            xT_r = xT_d.ap().rearrange("(c p) t -> p c t", p=128)

            # PSUM: pjps(2) + sps(2x2banks) + ops(2) = 8 banks
            pjps = _ctx.enter_context(tc.tile_pool(name="pjps", bufs=2, space="PSUM"))
            sps = _ctx.enter_context(tc.tile_pool(name="sps", bufs=2, space="PSUM"))
            ops = _ctx.enter_context(tc.tile_pool(name="ops", bufs=2, space="PSUM"))

            # PE warm-up: covers the first x-chunk DMA; ramps the PE p-state
            warm = sb.tile([128, TCH], BF16, name="warm")
            nc.vector.memset(warm, 0.0)
            wps = pjps.tile([128, TCH], F32, tag="pj", name="wps")
            for wi in range(10):
                nc.tensor.matmul(
                    wps[:], warm[:, 0:128], warm[:],
                    start=(wi == 0), stop=(wi == 9),
                )

            # ---------------- projection pieces ----------------
            def gen_proj_pieces(tci, defer_dma=False):
                """Return 8 piece thunks (plus a leading xt-DMA-issue thunk
                when defer_dma); with defer_dma=False the DMA is issued now."""
                tsl = slice(tci * TCH, (tci + 1) * TCH)
                xt = xtp.tile([128, DC, TCH], BF16, tag="xt", name=f"xt{tci}")

                def issue_xt():
                    nc.sync.dma_start(xt[:, 0:4, :], xT_r[:, 0:4, tsl])
                    nc.sync.dma_start(xt[:, 4:8, :], xT_r[:, 4:8, tsl])

                if not defer_dma:
                    issue_xt()

                def qk_piece(w_sb, dst8, p, pi):
                    is_q = dst8 is qt8

                    def run():
                        ps = pjps.tile([128, TCH], F32, tag="pj", name=f"pj{tci}_{pi}")
                        for dc in range(DC):
                            nc.tensor.matmul(
                                ps[:],
                                w_sb[:, dc, p * 128 : (p + 1) * 128],
                                xt[:, dc, :],
                                start=(dc == 0),
                                stop=(dc == DC - 1),
                            )
                        qkbf = ropep.tile([128, TCH], BF16, tag="qkbf", name=f"qb{tci}_{pi}")
                        nc.vector.tensor_copy(qkbf[:], ps[:])
                        t1 = ropep.tile([128, TCH], BF16, tag="t1", name=f"t1_{tci}_{pi}")
                        t2 = ropep.tile([128, TCH], BF16, tag="t2", name=f"t2_{tci}_{pi}")
                        swt = ropep.tile([128, TCH], BF16, tag="swt", name=f"sw{tci}_{pi}")
                        nc.vector.tensor_mul(t1[:], qkbf[:], cs1[:, tsl])
                        nc.vector.tensor_mul(t2[:], qkbf[:], cs2[:, tsl])
                        swring = nc.sync if is_q else nc.gpsimd
                        for s in range(4):
                            swring.dma_start(
                                swt[s * 32 : (s + 1) * 32, :],
                                t2[(s ^ 1) * 32 : ((s ^ 1) + 1) * 32, :],
                            )
                        radd = ropep.tile([128, TCH], FP8, tag="radd", name=f"ra{tci}_{pi}")
                        nc.vector.tensor_add(radd[:], t1[:], swt[:])
                        # fold into DoubleRow layout: head 2p -> base 0 of
                        # tile p, head 2p+1 -> base 32; dh = slot*32 + part
                        d8 = dst8[p]
                        r1, r2 = (nc.sync, nc.gpsimd) if is_q else (nc.gpsimd, nc.sync)
                        r1.dma_start(d8[0:32, 0, tsl], radd[0:32, :])
                        r1.dma_start(d8[0:32, 1, tsl], radd[32:64, :])
                        r2.dma_start(d8[32:64, 0, tsl], radd[64:96, :])
                        r2.dma_start(d8[32:64, 1, tsl], radd[96:128, :])
                    return run

                def v_piece(tt):
                    def run():
                        gt = tci * 4 + tt
                        ps = pjps.tile([128, TCH], F32, tag="pj", name=f"pjv{gt}")
                        for dc in range(DC):
                            nc.tensor.matmul(
                                ps[:, : HPC * DH],
                                xt[:, dc, tt * 128 : (tt + 1) * 128],
                                wv[:, dc, :],
                                start=(dc == 0),
                                stop=(dc == DC - 1),
                            )
                        nc.vector.tensor_copy(
                            vaug[:, gt, :, 0:DH],
                            ps[:, : HPC * DH].rearrange("p (h d) -> p h d", h=HPC),
                        )
                    return run

                pieces = []
                pi = 0
                for w_sb, dst8 in ((wq, qt8), (wk, kt8)):
                    for p in range(2):
                        pieces.append(qk_piece(w_sb, dst8, p, pi))
                        pi += 1
                for tt in range(4):
                    pieces.append(v_piece(tt))
                if defer_dma:
                    pieces.insert(0, issue_xt)
                return pieces

            # ---------------- out-projection pieces ----------------
            def gen_outproj_pieces(qc):
                def piece(tt):
                    def run():
                        gtt = qc * 4 + tt
                        ysb = ysbp.tile([128, 2 * TCH], F32, tag="ysb", name=f"ys{gtt}")
                        for ni in range(2):
                            ypsum = pjps.tile(
                                [128, TCH], F32, tag="pj", name=f"y{gtt}_{ni}"
                            )
                            for p2 in range(2):
                                nc.tensor.matmul(
                                    ypsum[:],
                                    ot[p2][:, gtt * 128 : (gtt + 1) * 128],
                                    wo[:, p2, ni * TCH : (ni + 1) * TCH],
                                    start=(p2 == 0),
                                    stop=(p2 == 1),
                                )
                            nc.vector.tensor_copy(
                                ysb[:, ni * TCH : (ni + 1) * TCH], ypsum[:]
                            )
                        nc.sync.dma_start(
                            y_d.ap()[gtt * 128 : (gtt + 1) * 128, :], ysb[:]
                        )
                    return run
                return [piece(tt) for tt in range(4)]

            # ---------------- flat attention pipeline ----------------
            # state shared across units
            pts = {}      # (qc,h) -> {kb: pt tile}
            opsums = {}   # (qc,h) -> psum tile
            oraws = {}    # (qc,h) -> oraw tile
            denps = {}    # (qc,hp) -> [2,TCH] den tile (partition-0 based)

            def emit_s_group(qc, h, g):
                """S matmuls (fp8 DoubleRow) + exp + mask for kb-pairs 2g, 2g+1."""
                nkt = 4 * qc + 4
                hp, par = divmod(h, 2)
                b32 = 32 * par
                for pairidx in (2 * g, 2 * g + 1):
                    kb = 2 * pairidx
                    if kb >= nkt:
                        continue
                    spt = sps.tile([128, 2, TCH], F32, tag="s", name=f"s{qc}_{h}_{kb}")
                    pt = ptp.tile([128, 2, TCH], BF16, tag="pt", name=f"pt{qc}_{h}_{kb}")
                    pts[(qc, h)][kb] = pt
                    offs = [max(0, 128 * (kb + j) - TCH * qc) for j in (0, 1)]
                    for j in (0, 1):
                        nc.tensor.matmul(
                            spt[:, j, offs[j] :],
                            kt8[hp][b32 : b32 + 32, :, (kb + j) * 128 : (kb + j + 1) * 128],
                            qt8[hp][b32 : b32 + 32, :, qc * TCH + offs[j] : (qc + 1) * TCH],
                            start=True,
                            stop=True,
                            perf_mode=mybir.MatmulPerfMode.DoubleRow,
                        )
                    if offs[1] > 0:
                        for j in (0, 1):
                            nc.scalar.activation(
                                pt[:, j, offs[j] :],
                                spt[:, j, offs[j] :],
                                EXP,
                                bias=0.0,
                                scale=0.125,
                            )
                    else:
                        nc.scalar.activation(
                            pt.rearrange("p a b -> p (a b)"),
                            spt.rearrange("p a b -> p (a b)"),
                            EXP,
                            bias=0.0,
                            scale=0.125,
                        )
                    for j in (0, 1):
                        kt = kb + j
                        if kt >= 4 * qc:
                            off = offs[j]
                            nc.gpsimd.affine_select(
                                out=pt[:, j, off:],
                                in_=pt[:, j, off:],
                                compare_op=mybir.AluOpType.is_ge,
                                fill=0.0,
                                base=0,
                                pattern=[[1, TCH - off]],
                                channel_multiplier=-1,
                            )

            def emit_av_group(qc, h, g):
                nkt = 4 * qc + 4
                for pairidx in (2 * g, 2 * g + 1):
                    kb = 2 * pairidx
                    if kb >= nkt:
                        continue
                    if kb == 0:
                        opsums[(qc, h)] = ops.tile(
                            [128, TCH], F32, tag="o", name=f"o{qc}_{h}"
                        )
                    pt = pts[(qc, h)][kb]
                    for j in (0, 1):
                        kt = kb + j
                        off = max(0, 128 * kt - TCH * qc)
                        nc.tensor.matmul(
                            opsums[(qc, h)][0 : DH + 1, off:],
                            vaug[:, kt, h, :],
                            pt[:, j, off:],
                            start=(kt == 0),
                            stop=(kt == nkt - 1),
                        )

            def emit_evac(qc, h):
                hp, par = divmod(h, 2)
                if (qc, hp) not in denps:
                    denps[(qc, hp)] = miscp.tile(
                        [2, TCH], F32, tag=f"den{hp}", name=f"den{qc}_{hp}"
                    )
                oraw = orawp.tile([128, TCH], F32, tag="oraw", name=f"or{qc}_{h}")
                oraws[(qc, h)] = oraw
                nc.vector.tensor_copy(oraw[0 : DH + 1, :], opsums[(qc, h)][0 : DH + 1, :])
                nc.sync.dma_start(denps[(qc, hp)][par : par + 1, :], oraw[DH : DH + 1, :])

            def emit_norm_pair(qc, hp):
                """Fast approx reciprocal, matmul-broadcast to 128 partitions,
                then scale into ot. ot layout is par-swapped: partitions 0-63
                hold the odd head of the pair (DVE-direct write), 64-127 the
                even head (via DMA). Emitted per pair, as soon as that pair's
                AV is done, so only the last pair sits on the kernel tail."""
                qsl = slice(qc * TCH, (qc + 1) * TCH)
                denr = miscp.tile([2, TCH], F32, tag=f"denr{hp}", name=f"dr{qc}_{hp}")
                nc.vector.reciprocal_approx_fast(out=denr[:], in_=denps[(qc, hp)][:])
                bcps = ops.tile([128, TCH], F32, tag="o", name=f"bc{qc}_{hp}")
                nc.tensor.matmul(bcps[:], sel[hp][:], denr[:], start=True, stop=True)
                h_hi, h_lo = 2 * hp + 1, 2 * hp
                nc.vector.tensor_mul(
                    ot[hp][0:64, qsl], oraws[(qc, h_hi)][0:64, :], bcps[0:64, :]
                )
                tmpo = miscp.tile([64, TCH], BF16, tag="tmpo", name=f"tp{qc}_{hp}")
                nc.vector.tensor_mul(
                    tmpo[:], oraws[(qc, h_lo)][0:64, :], bcps[64:128, :]
                )
                nc.sync.dma_start(ot[hp][64:128, qsl], tmpo[:])

            # ---------------- emission schedule ----------------
            p0 = gen_proj_pieces(0)
            p1 = gen_proj_pieces(1, defer_dma=True)
            # stagger the remaining preloads between proj(0) pieces so the
            # startup DMA bandwidth goes to wq+xt0 first
            order01 = (
                [p0[0], p0[1], p0[2], load_wv, p1[0]]
                + p0[3:]
                + p1[1:]
            )
            for f in order01:
                f()

            units = [(qc, h) for qc in CHUNK_ORDER for h in range(HPC)]
            # filler pieces attached per unit (consumed one per S-group slot)
            unit_fillers = {i: [] for i in range(len(units))}

            def attach(fillers, lo, hi):
                n = hi - lo
                for i, f in enumerate(fillers):
                    unit_fillers[lo + i * n // len(fillers)].append(f)

            attach(gen_proj_pieces(2, defer_dma=True), 0, 4)  # during chunk 1

            def load_wo():
                nc.gpsimd.dma_start(
                    wo[:], wo_d.ap().rearrange("(c p) n -> p c n", p=128)
                )

            # keys must come AFTER the unit that emits the norm the pieces read
            # (chunk order 1,2,3,0: norm(1)@ui4, norm(2)@ui8, norm(3)@ui12)
            lazy_fillers = {
                0: lambda: [load_wo, load_sel],
                4: lambda: gen_proj_pieces(3, defer_dma=True),  # chunk 2 units
                8: lambda: gen_outproj_pieces(1),   # chunk 3 units
                10: lambda: gen_outproj_pieces(2),
                13: lambda: gen_outproj_pieces(3),  # chunk 0 units
            }

            for ui, (qc, h) in enumerate(units):
                if ui in lazy_fillers:
                    pieces = lazy_fillers[ui]()
                    hi = min(ui + 4, len(units))
                    attach(pieces, ui, hi)
                pts[(qc, h)] = {}
                nkt = 4 * qc + 4
                ngroups = (nkt // 2 + 1) // 2  # S groups of 2 pairs
                prev = units[ui - 1] if ui > 0 else None
                pgroups = ((4 * prev[0] + 4) // 2 + 1) // 2 if prev else 0
                fill = unit_fillers[ui]
                for g in range(max(ngroups, pgroups)):
                    if fill:
                        fill.pop(0)()
                    if g < ngroups:
                        emit_s_group(qc, h, g)
                    if prev and g < pgroups:
                        emit_av_group(prev[0], prev[1], g)
                for f in fill:
                    f()
                fill.clear()
                if prev:
                    emit_evac(prev[0], prev[1])
                    if prev[1] == 1:
                        emit_norm_pair(prev[0], 0)
                    elif prev[1] == 3:
                        emit_norm_pair(prev[0], 1)

            # drain: AV + norm of the final unit, then outproj of last chunk
            qc_l, h_l = units[-1]
            for g in range(((4 * qc_l + 4) // 2 + 1) // 2):
                emit_av_group(qc_l, h_l, g)
            emit_evac(qc_l, h_l)
            emit_norm_pair(qc_l, 1)
            for f in gen_outproj_pieces(qc_l):
                f()
    nc.compile()
    return nc


_NC_CACHE = []


def _get_nc():
    if not _NC_CACHE:
        _NC_CACHE.append(_build())
    return _NC_CACHE[0]


_LAST_RESULTS = []  # stashed BassKernelResults for test harness introspection


def _wo_rows_parswap(Wout_rows):
    """Reorder the 256 Wout rows so each 128-row pair block is [odd-head 64 | even-head 64]."""
    out = np.empty_like(Wout_rows)
    for hp in range(2):
        blk = Wout_rows[hp * 128 : (hp + 1) * 128]
        out[hp * 128 : hp * 128 + 64] = blk[64:128]
        out[hp * 128 + 64 : (hp + 1) * 128] = blk[0:64]
    return out


def kernel(x, Wqkv, Wout, _trace=False, **_trace_kwargs):
    x = np.asarray(x, dtype=np.float32)
    Wqkv = np.asarray(Wqkv, dtype=np.float32)
    Wout = np.asarray(Wout, dtype=np.float32)

    nc = _get_nc()
    in_maps = []
    for c in range(NCORES):
        b, g = divmod(c, HPC)
        cols = slice(g * HPC * DH, (g + 1) * HPC * DH)
        rows = slice(g * HPC * DH, (g + 1) * HPC * DH)
        bf = ml_dtypes.bfloat16
        in_maps.append(
            {
                "xT": np.ascontiguousarray(x[b].T.astype(bf)),
                "wq": np.ascontiguousarray(Wqkv[:, 0:D][:, cols].astype(bf)),
                "wk": np.ascontiguousarray(Wqkv[:, D : 2 * D][:, cols].astype(bf)),
                "wv": np.ascontiguousarray(Wqkv[:, 2 * D : 3 * D][:, cols].astype(bf)),
                "wo": np.ascontiguousarray(
                    _wo_rows_parswap(Wout[rows, :]).astype(bf)
                ),
            }
        )

    res = run_bass_kernel_spmd(
        nc, in_maps, core_ids=list(range(NCORES)), trace=_trace, **_trace_kwargs
    )
    _LAST_RESULTS.clear()
    _LAST_RESULTS.append(res)

    out = np.zeros((B, T, D), dtype=np.float32)
    for c in range(NCORES):
        b = c // HPC
        out[b] += res.results[c]["y"]
    return out


# revision 54
# speedup vs baseline: 1.0393x; 1.0393x over previous
"""Trainium2 Bass kernel for EnhancedAttention (B=2, T=2048, D=1024, H=16, DH=64).

Sharding: 8 cores = 2 batches x 4 head-groups (4 heads each). No collectives;
each core computes a partial out-projection and the host sums the 4 partials
per batch.

v3: flat zippered attention pipeline across (chunk, head) units so the PE
stream stays dense (clock ramped):
  - unit (qc,h)'s S-matmul groups interleave with unit-prev's AV groups,
    including across chunk boundaries
  - proj(c+1)/outproj(c') pieces fill remaining slack
  - S/AV emitted in groups of 2 kb-pairs to amortize PE row-config switches
  - denominator reciprocal batched [4,512]; broadcast via tiny matmuls
  - ot partition layout par-swapped (host reorders Wout rows) so the last
    head's normalize writes SBUF directly
"""
import os
import sys

for _p in ("/opt/trn_rl_repo", "/root/.axon_site/_ro/trn_rl_repo"):
    if os.path.isdir(_p) and _p not in sys.path:
        sys.path.append(_p)

import ml_dtypes
import numpy as np

import concourse.bass as bass  # noqa: F401
import concourse.tile as tile
from concourse import bacc, mybir
from concourse.bass_utils import run_bass_kernel_spmd

B, T, D = 2, 2048, 1024
H, DH = 16, 64
HPC = 4  # heads per core
NCORES = 8
ROPE_THETA = 10000.0

F32 = mybir.dt.float32
BF16 = mybir.dt.bfloat16
FP8 = mybir.dt.float8e4

TCH = 512  # t-chunk (q-chunk) size
TC = T // TCH  # 4
DC = D // 128  # 8 contraction chunks
NKT = T // 128  # 16 k-tiles

CHUNK_ORDER = [1, 2, 3, 0]


def _rope_tables():
    inv = 1.0 / (ROPE_THETA ** (np.arange(0, DH, 2, dtype=np.float64) / DH))
    f = np.arange(T, dtype=np.float64)[:, None] * inv[None, :]  # [T, 32]
    import ml_dtypes as _md
    cos = np.cos(f).T.astype(_md.bfloat16)  # [32, T]
    sin = np.sin(f).T.astype(_md.bfloat16)
    cs1 = np.ascontiguousarray(np.tile(cos, (4, 1)))  # [128, T]
    cs2 = np.ascontiguousarray(np.concatenate([sin, -sin, sin, -sin], axis=0))
    return cs1, cs2


def _build():
    nc = bacc.Bacc("TRN2", target_bir_lowering=False, debug=False, num_devices=NCORES)
    xT_d = nc.dram_tensor("xT", [D, T], BF16, kind="ExternalInput")
    wq_d = nc.dram_tensor("wq", [D, HPC * DH], BF16, kind="ExternalInput")
    wk_d = nc.dram_tensor("wk", [D, HPC * DH], BF16, kind="ExternalInput")
    wv_d = nc.dram_tensor("wv", [D, HPC * DH], BF16, kind="ExternalInput")
    wo_d = nc.dram_tensor("wo", [HPC * DH, D], BF16, kind="ExternalInput")
    y_d = nc.dram_tensor("y", [T, D], F32, kind="ExternalOutput")

    cs1_np, cs2_np = _rope_tables()
    # unique rows only: cs1 is [32,T] tiled 4x, cs2 is [64,T] tiled 2x
    cs1_d = nc.inline_tensor(np.ascontiguousarray(cs1_np[:32]), "cs1")
    cs2_d = nc.inline_tensor(np.ascontiguousarray(cs2_np[:64]), "cs2")

    EXP = mybir.ActivationFunctionType.Exp
    COPYF = mybir.ActivationFunctionType.Copy

    import contextlib
    with tile.TileContext(nc) as tc:
        with (
            contextlib.ExitStack() as _ctx,
            tc.tile_pool(name="sb", bufs=1) as sb,
            tc.tile_pool(name="xtp", bufs=2) as xtp,
            tc.tile_pool(name="ropep", bufs=2) as ropep,
            tc.tile_pool(name="ptp", bufs=18) as ptp,
            tc.tile_pool(name="orawp", bufs=6) as orawp,
            tc.tile_pool(name="miscp", bufs=2) as miscp,
            tc.tile_pool(name="ysbp", bufs=2) as ysbp,
        ):
            wq = sb.tile([128, DC, HPC * DH], BF16)
            wk = sb.tile([128, DC, HPC * DH], BF16)
            wv = sb.tile([128, DC, HPC * DH], BF16)
            wo = sb.tile([128, 2, D], BF16)
            cs1 = sb.tile([128, T], BF16)
            cs2 = sb.tile([128, T], BF16)
            qt = [sb.tile([128, T], BF16, tag=f"qt{p}", name=f"qt{p}") for p in range(2)]
            ktt = [sb.tile([128, T], BF16, tag=f"kt{p}", name=f"kt{p}") for p in range(2)]
            # constant causal mask for diagonal k-tiles: keep col >= row
            maskt = sb.tile([128, TCH], BF16, name="maskt")
            mask_np = (np.arange(TCH)[None, :] >= np.arange(128)[:, None])
            mask_d = nc.inline_tensor(
                np.ascontiguousarray(mask_np.astype(ml_dtypes.bfloat16)), "maskc"
            )
            vaug = sb.tile([128, NKT, HPC, DH + 1], BF16)
            ot = [sb.tile([128, T], BF16, tag=f"ot{p}", name=f"ot{p}") for p in range(2)]
            # selector constants for matmul-based denominator broadcast:
            # bcps[hp] partitions 0-63 get head 2hp+1's recip (par-swapped ot
            # layout), partitions 64-127 get head 2hp's
            sel = [sb.tile([2, 128], F32, tag=f"sel{hp}", name=f"sel{hp}") for hp in range(2)]

            def load_sel():
                for hp in range(2):
                    sel_np = np.zeros((2, 128), dtype=np.float32)
                    sel_np[1, 0:64] = 1.0
                    sel_np[0, 64:128] = 1.0
                    sel_d = nc.inline_tensor(sel_np, f"selc{hp}")
                    nc.scalar.dma_start(sel[hp][:], sel_d.ap())

            # preloads in need-order: wq first (piece 1), cs (rope, ~12us),
            # wk/wv on the gpsimd ring; wo/sel deferred to unit-0 fillers
            nc.scalar.dma_start(wq[:], wq_d.ap().rearrange("(c p) n -> p c n", p=128))
            nc.scalar.dma_start(cs1[0:32, :], cs1_d.ap())
            nc.scalar.dma_start(cs2[0:64, :], cs2_d.ap())
            for rep in range(1, 4):
                nc.scalar.dma_start(cs1[rep * 32 : (rep + 1) * 32, :], cs1[0:32, :])
            nc.scalar.dma_start(cs2[64:128, :], cs2[0:64, :])
            nc.gpsimd.dma_start(wk[:], wk_d.ap().rearrange("(c p) n -> p c n", p=128))
            nc.gpsimd.dma_start(maskt[:], mask_d.ap())

            def load_wv():
                # sits behind piece-2's k-swap DMAs on the gpsimd queue, so
                # it doesn't steal startup bandwidth from wq/xt0/wk
                nc.gpsimd.dma_start(wv[:], wv_d.ap().rearrange("(c p) n -> p c n", p=128))

            nc.vector.memset(vaug[:, :, :, DH : DH + 1], 1.0)

            xT_r = xT_d.ap().rearrange("(c p) t -> p c t", p=128)

            # PSUM: pjps(2) + sps(2x2banks) + ops(2) = 8 banks
            pjps = _ctx.enter_context(tc.tile_pool(name="pjps", bufs=2, space="PSUM"))
            sps = _ctx.enter_context(tc.tile_pool(name="sps", bufs=2, space="PSUM"))
            ops = _ctx.enter_context(tc.tile_pool(name="ops", bufs=2, space="PSUM"))

            # PE warm-up: covers the first x-chunk DMA; ramps the PE p-state
            warm = sb.tile([128, TCH], BF16, name="warm")
            nc.vector.memset(warm, 0.0)
            wps = pjps.tile([128, TCH], F32, tag="pj", name="wps")
            for wi in range(10):
                nc.tensor.matmul(
                    wps[:], warm[:, 0:128], warm[:],
                    start=(wi == 0), stop=(wi == 9),
                )

            # ---------------- projection pieces ----------------
            def gen_proj_pieces(tci, defer_dma=False):
                """Return 8 piece thunks (plus a leading xt-DMA-issue thunk
                when defer_dma); with defer_dma=False the DMA is issued now."""
                tsl = slice(tci * TCH, (tci + 1) * TCH)
                xt = xtp.tile([128, DC, TCH], BF16, tag="xt", name=f"xt{tci}")

                def issue_xt():
                    nc.sync.dma_start(xt[:, 0:4, :], xT_r[:, 0:4, tsl])
                    nc.sync.dma_start(xt[:, 4:8, :], xT_r[:, 4:8, tsl])

                if not defer_dma:
                    issue_xt()

                def qk_piece(w_sb, dest, p, pi):
                    is_q = dest is qt

                    def run():
                        ps = pjps.tile([128, TCH], F32, tag="pj", name=f"pj{tci}_{pi}")
                        for dc in range(DC):
                            nc.tensor.matmul(
                                ps[:],
                                w_sb[:, dc, p * 128 : (p + 1) * 128],
                                xt[:, dc, :],
                                start=(dc == 0),
                                stop=(dc == DC - 1),
                            )
                        qkbf = ropep.tile([128, TCH], BF16, tag="qkbf", name=f"qb{tci}_{pi}")
                        nc.vector.tensor_copy(qkbf[:], ps[:])
                        t1 = ropep.tile([128, TCH], BF16, tag="t1", name=f"t1_{tci}_{pi}")
                        t2 = ropep.tile([128, TCH], BF16, tag="t2", name=f"t2_{tci}_{pi}")
                        swt = ropep.tile([128, TCH], BF16, tag="swt", name=f"sw{tci}_{pi}")
                        nc.vector.tensor_mul(t1[:], qkbf[:], cs1[:, tsl])
                        nc.vector.tensor_mul(t2[:], qkbf[:], cs2[:, tsl])
                        swring = nc.sync if is_q else nc.gpsimd
                        for s in range(4):
                            swring.dma_start(
                                swt[s * 32 : (s + 1) * 32, :],
                                t2[(s ^ 1) * 32 : ((s ^ 1) + 1) * 32, :],
                            )
                        nc.vector.tensor_add(dest[p][:, tsl], t1[:], swt[:])
                    return run

                def v_piece(tt):
                    def run():
                        gt = tci * 4 + tt
                        ps = pjps.tile([128, TCH], F32, tag="pj", name=f"pjv{gt}")
                        for dc in range(DC):
                            nc.tensor.matmul(
                                ps[:, : HPC * DH],
                                xt[:, dc, tt * 128 : (tt + 1) * 128],
                                wv[:, dc, :],
                                start=(dc == 0),
                                stop=(dc == DC - 1),
                            )
                        nc.vector.tensor_copy(
                            vaug[:, gt, :, 0:DH],
                            ps[:, : HPC * DH].rearrange("p (h d) -> p h d", h=HPC),
                        )
                    return run

                pieces = []
                pi = 0
                for w_sb, dest in ((wq, qt), (wk, ktt)):
                    for p in range(2):
                        pieces.append(qk_piece(w_sb, dest, p, pi))
                        pi += 1
                for tt in range(4):
                    pieces.append(v_piece(tt))
                if defer_dma:
                    pieces.insert(0, issue_xt)
                return pieces

            # ---------------- out-projection pieces ----------------
            def gen_outproj_pieces(qc):
                def piece(tt):
                    def run():
                        gtt = qc * 4 + tt
                        ysb = ysbp.tile([128, 2 * TCH], F32, tag="ysb", name=f"ys{gtt}")
                        for ni in range(2):
                            ypsum = pjps.tile(
                                [128, TCH], F32, tag="pj", name=f"y{gtt}_{ni}"
                            )
                            for p2 in range(2):
                                nc.tensor.matmul(
                                    ypsum[:],
                                    ot[p2][:, gtt * 128 : (gtt + 1) * 128],
                                    wo[:, p2, ni * TCH : (ni + 1) * TCH],
                                    start=(p2 == 0),
                                    stop=(p2 == 1),
                                )
                            nc.vector.tensor_copy(
                                ysb[:, ni * TCH : (ni + 1) * TCH], ypsum[:]
                            )
                        nc.sync.dma_start(
                            y_d.ap()[gtt * 128 : (gtt + 1) * 128, :], ysb[:]
                        )
                    return run
                return [piece(tt) for tt in range(4)]

            # ---------------- flat attention pipeline ----------------
            # state shared across units
            pts = {}      # (qc,h) -> {kb: pt tile}
            opsums = {}   # (qc,h) -> psum tile
            oraws = {}    # (qc,h) -> oraw tile
            denps = {}    # (qc,hp) -> [2,TCH] den tile (partition-0 based)

            def emit_s_group(qc, h, g):
                """S matmuls + exp + mask for kb-pairs 2g, 2g+1 of unit."""
                nkt = 4 * qc + 4
                hp, par = divmod(h, 2)
                qsl = slice(qc * TCH, (qc + 1) * TCH)
                qrh = qt[hp][par * 64 : par * 64 + 64, qsl]
                for pairidx in (2 * g, 2 * g + 1):
                    kb = 2 * pairidx
                    if kb >= nkt:
                        continue
                    spt = sps.tile([128, 2, TCH], F32, tag="s", name=f"s{qc}_{h}_{kb}")
                    pt = ptp.tile([128, 2, TCH], BF16, tag="pt", name=f"pt{qc}_{h}_{kb}")
                    pts[(qc, h)][kb] = pt
                    offs = [max(0, 128 * (kb + j) - TCH * qc) for j in (0, 1)]
                    for j in (0, 1):
                        nc.tensor.matmul(
                            spt[:, j, offs[j] :],
                            ktt[hp][
                                par * 64 : par * 64 + 64,
                                (kb + j) * 128 : (kb + j + 1) * 128,
                            ],
                            qrh[:, offs[j] :],
                            start=True,
                            stop=True,
                        )
                    if offs[1] > 0:
                        for j in (0, 1):
                            nc.scalar.activation(
                                pt[:, j, offs[j] :],
                                spt[:, j, offs[j] :],
                                EXP,
                                bias=0.0,
                                scale=0.125,
                            )
                    else:
                        nc.scalar.activation(
                            pt.rearrange("p a b -> p (a b)"),
                            spt.rearrange("p a b -> p (a b)"),
                            EXP,
                            bias=0.0,
                            scale=0.125,
                        )
                    for j in (0, 1):
                        kt = kb + j
                        if kt >= 4 * qc:
                            off = offs[j]
                            if h % 2 == 1:
                                # causal mask as a DVE multiply with the
                                # constant triangular tile (gpsimd relief)
                                nc.vector.tensor_mul(
                                    pt[:, j, off:],
                                    pt[:, j, off:],
                                    maskt[:, : TCH - off],
                                )
                            else:
                                nc.gpsimd.affine_select(
                                    out=pt[:, j, off:],
                                    in_=pt[:, j, off:],
                                    compare_op=mybir.AluOpType.is_ge,
                                    fill=0.0,
                                    base=0,
                                    pattern=[[1, TCH - off]],
                                    channel_multiplier=-1,
                                )

            def emit_av_group(qc, h, g):
                nkt = 4 * qc + 4
                for pairidx in (2 * g, 2 * g + 1):
                    kb = 2 * pairidx
                    if kb >= nkt:
                        continue
                    if kb == 0:
                        opsums[(qc, h)] = ops.tile(
                            [128, TCH], F32, tag="o", name=f"o{qc}_{h}"
                        )
                    pt = pts[(qc, h)][kb]
                    for j in (0, 1):
                        kt = kb + j
                        off = max(0, 128 * kt - TCH * qc)
                        nc.tensor.matmul(
                            opsums[(qc, h)][0 : DH + 1, off:],
                            vaug[:, kt, h, :],
                            pt[:, j, off:],
                            start=(kt == 0),
                            stop=(kt == nkt - 1),
                        )

            def emit_evac(qc, h):
                hp, par = divmod(h, 2)
                if (qc, hp) not in denps:
                    denps[(qc, hp)] = miscp.tile(
                        [2, TCH], F32, tag=f"den{hp}", name=f"den{qc}_{hp}"
                    )
                oraw = orawp.tile([128, TCH], F32, tag="oraw", name=f"or{qc}_{h}")
                oraws[(qc, h)] = oraw
                nc.vector.tensor_copy(oraw[0 : DH + 1, :], opsums[(qc, h)][0 : DH + 1, :])
                nc.sync.dma_start(denps[(qc, hp)][par : par + 1, :], oraw[DH : DH + 1, :])

            def emit_norm_pair(qc, hp):
                """Fast approx reciprocal, matmul-broadcast to 128 partitions,
                then scale into ot. ot layout is par-swapped: partitions 0-63
                hold the odd head of the pair (DVE-direct write), 64-127 the
                even head (via DMA). Emitted per pair, as soon as that pair's
                AV is done, so only the last pair sits on the kernel tail."""
                qsl = slice(qc * TCH, (qc + 1) * TCH)
                denr = miscp.tile([2, TCH], F32, tag=f"denr{hp}", name=f"dr{qc}_{hp}")
                nc.vector.reciprocal_approx_fast(out=denr[:], in_=denps[(qc, hp)][:])
                bcps = ops.tile([128, TCH], F32, tag="o", name=f"bc{qc}_{hp}")
                nc.tensor.matmul(bcps[:], sel[hp][:], denr[:], start=True, stop=True)
                h_hi, h_lo = 2 * hp + 1, 2 * hp
                nc.vector.tensor_mul(
                    ot[hp][0:64, qsl], oraws[(qc, h_hi)][0:64, :], bcps[0:64, :]
                )
                tmpo = miscp.tile([64, TCH], BF16, tag="tmpo", name=f"tp{qc}_{hp}")
                nc.vector.tensor_mul(
                    tmpo[:], oraws[(qc, h_lo)][0:64, :], bcps[64:128, :]
                )
                nc.sync.dma_start(ot[hp][64:128, qsl], tmpo[:])

            # ---------------- emission schedule ----------------
            p0 = gen_proj_pieces(0)
            p1 = gen_proj_pieces(1, defer_dma=True)
            # stagger the remaining preloads between proj(0) pieces so the
            # startup DMA bandwidth goes to wq+xt0 first
            order01 = (
                [p0[0], p0[1], p0[2], load_wv, p1[0]]
                + p0[3:]
                + p1[1:]
            )
            for f in order01:
                f()

            units = [(qc, h) for qc in CHUNK_ORDER for h in range(HPC)]
            # filler pieces attached per unit (consumed one per S-group slot)
            unit_fillers = {i: [] for i in range(len(units))}

            def attach(fillers, lo, hi):
                n = hi - lo
                for i, f in enumerate(fillers):
                    unit_fillers[lo + i * n // len(fillers)].append(f)

            attach(gen_proj_pieces(2, defer_dma=True), 0, 4)  # during chunk 1

            def load_wo():
                nc.gpsimd.dma_start(
                    wo[:], wo_d.ap().rearrange("(c p) n -> p c n", p=128)
                )

            # keys must come AFTER the unit that emits the norm the pieces read
            # (chunk order 1,2,3,0: norm(1)@ui4, norm(2)@ui8, norm(3)@ui12)
            lazy_fillers = {
                0: lambda: [load_wo, load_sel],
                4: lambda: gen_proj_pieces(3, defer_dma=True),  # chunk 2 units
                8: lambda: gen_outproj_pieces(1),   # chunk 3 units
                10: lambda: gen_outproj_pieces(2),
                13: lambda: gen_outproj_pieces(3),  # chunk 0 units
            }

            for ui, (qc, h) in enumerate(units):
                if ui in lazy_fillers:
                    pieces = lazy_fillers[ui]()
                    hi = min(ui + 4, len(units))
                    attach(pieces, ui, hi)
                pts[(qc, h)] = {}
                nkt = 4 * qc + 4
                ngroups = (nkt // 2 + 1) // 2  # S groups of 2 pairs
                prev = units[ui - 1] if ui > 0 else None
                pgroups = ((4 * prev[0] + 4) // 2 + 1) // 2 if prev else 0
                fill = unit_fillers[ui]
                for g in range(max(ngroups, pgroups)):
                    if fill:
                        fill.pop(0)()
                    if g < ngroups:
                        emit_s_group(qc, h, g)
                    if prev and g < pgroups:
                        emit_av_group(prev[0], prev[1], g)
                for f in fill:
                    f()
                fill.clear()
                if prev:
                    emit_evac(prev[0], prev[1])
                    if prev[1] == 1:
                        emit_norm_pair(prev[0], 0)
                    elif prev[1] == 3:
                        emit_norm_pair(prev[0], 1)

            # drain: AV + norm of the final unit, then outproj of last chunk
            qc_l, h_l = units[-1]
            for g in range(((4 * qc_l + 4) // 2 + 1) // 2):
                emit_av_group(qc_l, h_l, g)
            emit_evac(qc_l, h_l)
            emit_norm_pair(qc_l, 1)
            for f in gen_outproj_pieces(qc_l):
                f()
    nc.compile()
    return nc


_NC_CACHE = []


def _get_nc():
    if not _NC_CACHE:
        _NC_CACHE.append(_build())
    return _NC_CACHE[0]


_LAST_RESULTS = []  # stashed BassKernelResults for test harness introspection


def _wo_rows_parswap(Wout_rows):
    """Reorder the 256 Wout rows so each 128-row pair block is [odd-head 64 | even-head 64]."""
    out = np.empty_like(Wout_rows)
    for hp in range(2):
        blk = Wout_rows[hp * 128 : (hp + 1) * 128]
        out[hp * 128 : hp * 128 + 64] = blk[64:128]
        out[hp * 128 + 64 : (hp + 1) * 128] = blk[0:64]
    return out


def kernel(x, Wqkv, Wout, _trace=False, **_trace_kwargs):
    x = np.asarray(x, dtype=np.float32)
    Wqkv = np.asarray(Wqkv, dtype=np.float32)
    Wout = np.asarray(Wout, dtype=np.float32)

    nc = _get_nc()
    in_maps = []
    for c in range(NCORES):
        b, g = divmod(c, HPC)
        cols = slice(g * HPC * DH, (g + 1) * HPC * DH)
        rows = slice(g * HPC * DH, (g + 1) * HPC * DH)
        bf = ml_dtypes.bfloat16
        in_maps.append(
            {
                "xT": np.ascontiguousarray(x[b].T.astype(bf)),
                "wq": np.ascontiguousarray(Wqkv[:, 0:D][:, cols].astype(bf)),
                "wk": np.ascontiguousarray(Wqkv[:, D : 2 * D][:, cols].astype(bf)),
                "wv": np.ascontiguousarray(Wqkv[:, 2 * D : 3 * D][:, cols].astype(bf)),
                "wo": np.ascontiguousarray(
                    _wo_rows_parswap(Wout[rows, :]).astype(bf)
                ),
            }
        )

    res = run_bass_kernel_spmd(
        nc, in_maps, core_ids=list(range(NCORES)), trace=_trace, **_trace_kwargs
    )
    _LAST_RESULTS.clear()
    _LAST_RESULTS.append(res)

    out = np.zeros((B, T, D), dtype=np.float32)
    for c in range(NCORES):
        b = c // HPC
        out[b] += res.results[c]["y"]
    return out


# revision 57
# speedup vs baseline: 1.0454x; 1.0059x over previous
"""Trainium2 Bass kernel for EnhancedAttention (B=2, T=2048, D=1024, H=16, DH=64).

Sharding: 8 cores = 2 batches x 4 head-groups (4 heads each). No collectives;
each core computes a partial out-projection and the host sums the 4 partials
per batch.

v3: flat zippered attention pipeline across (chunk, head) units so the PE
stream stays dense (clock ramped):
  - unit (qc,h)'s S-matmul groups interleave with unit-prev's AV groups,
    including across chunk boundaries
  - proj(c+1)/outproj(c') pieces fill remaining slack
  - S/AV emitted in groups of 2 kb-pairs to amortize PE row-config switches
  - denominator reciprocal batched [4,512]; broadcast via tiny matmuls
  - ot partition layout par-swapped (host reorders Wout rows) so the last
    head's normalize writes SBUF directly
"""
import os
import sys

for _p in ("/opt/trn_rl_repo", "/root/.axon_site/_ro/trn_rl_repo"):
    if os.path.isdir(_p) and _p not in sys.path:
        sys.path.append(_p)

import ml_dtypes
import numpy as np

import concourse.bass as bass  # noqa: F401
import concourse.tile as tile
from concourse import bacc, mybir
from concourse.bass_utils import run_bass_kernel_spmd

B, T, D = 2, 2048, 1024
H, DH = 16, 64
HPC = 4  # heads per core
NCORES = 8
ROPE_THETA = 10000.0

F32 = mybir.dt.float32
BF16 = mybir.dt.bfloat16
FP8 = mybir.dt.float8e4

TCH = 512  # t-chunk (q-chunk) size
TC = T // TCH  # 4
DC = D // 128  # 8 contraction chunks
NKT = T // 128  # 16 k-tiles

CHUNK_ORDER = [1, 2, 3, 0]


def _rope_tables():
    inv = 1.0 / (ROPE_THETA ** (np.arange(0, DH, 2, dtype=np.float64) / DH))
    f = np.arange(T, dtype=np.float64)[:, None] * inv[None, :]  # [T, 32]
    import ml_dtypes as _md
    cos = np.cos(f).T.astype(_md.bfloat16)  # [32, T]
    sin = np.sin(f).T.astype(_md.bfloat16)
    cs1 = np.ascontiguousarray(np.tile(cos, (4, 1)))  # [128, T]
    cs2 = np.ascontiguousarray(np.concatenate([sin, -sin, sin, -sin], axis=0))
    return cs1, cs2


def _build():
    nc = bacc.Bacc("TRN2", target_bir_lowering=False, debug=False, num_devices=NCORES)
    xT_d = nc.dram_tensor("xT", [D, T], BF16, kind="ExternalInput")
    wq_d = nc.dram_tensor("wq", [D, HPC * DH], BF16, kind="ExternalInput")
    wk_d = nc.dram_tensor("wk", [D, HPC * DH], BF16, kind="ExternalInput")
    wv_d = nc.dram_tensor("wv", [D, HPC * DH], BF16, kind="ExternalInput")
    wo_d = nc.dram_tensor("wo", [HPC * DH, D], BF16, kind="ExternalInput")
    y_d = nc.dram_tensor("y", [T, D], F32, kind="ExternalOutput")

    cs1_np, cs2_np = _rope_tables()
    # unique rows only: cs1 is [32,T] tiled 4x, cs2 is [64,T] tiled 2x
    cs1_d = nc.inline_tensor(np.ascontiguousarray(cs1_np[:32]), "cs1")
    cs2_d = nc.inline_tensor(np.ascontiguousarray(cs2_np[:64]), "cs2")

    EXP = mybir.ActivationFunctionType.Exp
    COPYF = mybir.ActivationFunctionType.Copy

    import contextlib
    with tile.TileContext(nc) as tc:
        with (
            contextlib.ExitStack() as _ctx,
            tc.tile_pool(name="sb", bufs=1) as sb,
            tc.tile_pool(name="xtp", bufs=2) as xtp,
            tc.tile_pool(name="ropep", bufs=2) as ropep,
            tc.tile_pool(name="ptp", bufs=18) as ptp,
            tc.tile_pool(name="orawp", bufs=6) as orawp,
            tc.tile_pool(name="miscp", bufs=2) as miscp,
            tc.tile_pool(name="ysbp", bufs=2) as ysbp,
        ):
            wq = sb.tile([128, DC, HPC * DH], BF16)
            wk = sb.tile([128, DC, HPC * DH], BF16)
            wv = sb.tile([128, DC, HPC * DH], BF16)
            wo = sb.tile([128, 2, D], BF16)
            cs1 = sb.tile([128, T], BF16)
            cs2 = sb.tile([128, T], BF16)
            qt = [sb.tile([128, T], BF16, tag=f"qt{p}", name=f"qt{p}") for p in range(2)]
            ktt = [sb.tile([128, T], BF16, tag=f"kt{p}", name=f"kt{p}") for p in range(2)]
            # constant causal mask for diagonal k-tiles: keep col >= row
            maskt = sb.tile([128, TCH], BF16, name="maskt")
            mask_np = (np.arange(TCH)[None, :] >= np.arange(128)[:, None])
            mask_d = nc.inline_tensor(
                np.ascontiguousarray(mask_np.astype(ml_dtypes.bfloat16)), "maskc"
            )
            vaug = sb.tile([128, NKT, HPC, DH + 1], BF16)
            ot = [sb.tile([128, T], BF16, tag=f"ot{p}", name=f"ot{p}") for p in range(2)]
            # selector constants for matmul-based denominator broadcast:
            # bcps[hp] partitions 0-63 get head 2hp+1's recip (par-swapped ot
            # layout), partitions 64-127 get head 2hp's
            sel = [sb.tile([2, 128], F32, tag=f"sel{hp}", name=f"sel{hp}") for hp in range(2)]

            def load_sel():
                for hp in range(2):
                    sel_np = np.zeros((2, 128), dtype=np.float32)
                    sel_np[1, 0:64] = 1.0
                    sel_np[0, 64:128] = 1.0
                    sel_d = nc.inline_tensor(sel_np, f"selc{hp}")
                    nc.scalar.dma_start(sel[hp][:], sel_d.ap())

            # preloads in need-order: wq first (piece 1), cs (rope, ~12us),
            # wk/wv on the gpsimd ring; wo/sel deferred to unit-0 fillers
            nc.scalar.dma_start(wq[:], wq_d.ap().rearrange("(c p) n -> p c n", p=128))
            nc.scalar.dma_start(cs1[0:32, :], cs1_d.ap())
            nc.scalar.dma_start(cs2[0:64, :], cs2_d.ap())
            for rep in range(1, 4):
                nc.scalar.dma_start(cs1[rep * 32 : (rep + 1) * 32, :], cs1[0:32, :])
            nc.scalar.dma_start(cs2[64:128, :], cs2[0:64, :])
            nc.gpsimd.dma_start(wk[:], wk_d.ap().rearrange("(c p) n -> p c n", p=128))
            nc.gpsimd.dma_start(maskt[:], mask_d.ap())

            def load_wv():
                # sits behind piece-2's k-swap DMAs on the gpsimd queue, so
                # it doesn't steal startup bandwidth from wq/xt0/wk
                nc.gpsimd.dma_start(wv[:], wv_d.ap().rearrange("(c p) n -> p c n", p=128))

            nc.vector.memset(vaug[:, :, :, DH : DH + 1], 1.0)

            xT_r = xT_d.ap().rearrange("(c p) t -> p c t", p=128)

            # PSUM: pjps(2) + sps(2x2banks) + ops(2) = 8 banks
            pjps = _ctx.enter_context(tc.tile_pool(name="pjps", bufs=2, space="PSUM"))
            sps = _ctx.enter_context(tc.tile_pool(name="sps", bufs=2, space="PSUM"))
            ops = _ctx.enter_context(tc.tile_pool(name="ops", bufs=2, space="PSUM"))

            # PE warm-up: covers the first x-chunk DMA; ramps the PE p-state
            warm = sb.tile([128, TCH], BF16, name="warm")
            nc.vector.memset(warm, 0.0)
            wps = pjps.tile([128, TCH], F32, tag="pj", name="wps")
            for wi in range(8):
                nc.tensor.matmul(
                    wps[:], warm[:, 0:128], warm[:],
                    start=(wi == 0), stop=(wi == 7),
                )

            # ---------------- projection pieces ----------------
            def gen_proj_pieces(tci, defer_dma=False):
                """Return 8 piece thunks (plus a leading xt-DMA-issue thunk
                when defer_dma); with defer_dma=False the DMA is issued now."""
                tsl = slice(tci * TCH, (tci + 1) * TCH)
                xt = xtp.tile([128, DC, TCH], BF16, tag="xt", name=f"xt{tci}")

                def issue_xt():
                    nc.sync.dma_start(xt[:, 0:4, :], xT_r[:, 0:4, tsl])
                    nc.sync.dma_start(xt[:, 4:8, :], xT_r[:, 4:8, tsl])

                if not defer_dma:
                    issue_xt()

                def qk_piece(w_sb, dest, p, pi):
                    is_q = dest is qt

                    def run():
                        ps = pjps.tile([128, TCH], F32, tag="pj", name=f"pj{tci}_{pi}")
                        for dc in range(DC):
                            nc.tensor.matmul(
                                ps[:],
                                w_sb[:, dc, p * 128 : (p + 1) * 128],
                                xt[:, dc, :],
                                start=(dc == 0),
                                stop=(dc == DC - 1),
                            )
                        qkbf = ropep.tile([128, TCH], BF16, tag="qkbf", name=f"qb{tci}_{pi}")
                        if tci < 2:
                            # scalar is idle at startup; keep the early DVE
                            # queue clear for the rope chain
                            nc.scalar.activation(qkbf[:], ps[:], COPYF, bias=0.0, scale=1.0)
                        else:
                            nc.vector.tensor_copy(qkbf[:], ps[:])
                        t1 = ropep.tile([128, TCH], BF16, tag="t1", name=f"t1_{tci}_{pi}")
                        t2 = ropep.tile([128, TCH], BF16, tag="t2", name=f"t2_{tci}_{pi}")
                        swt = ropep.tile([128, TCH], BF16, tag="swt", name=f"sw{tci}_{pi}")
                        nc.vector.tensor_mul(t1[:], qkbf[:], cs1[:, tsl])
                        nc.vector.tensor_mul(t2[:], qkbf[:], cs2[:, tsl])
                        swring = nc.sync if is_q else nc.gpsimd
                        for s in range(4):
                            swring.dma_start(
                                swt[s * 32 : (s + 1) * 32, :],
                                t2[(s ^ 1) * 32 : ((s ^ 1) + 1) * 32, :],
                            )
                        nc.vector.tensor_add(dest[p][:, tsl], t1[:], swt[:])
                    return run

                def v_piece(tt):
                    def run():
                        gt = tci * 4 + tt
                        ps = pjps.tile([128, TCH], F32, tag="pj", name=f"pjv{gt}")
                        for dc in range(DC):
                            nc.tensor.matmul(
                                ps[:, : HPC * DH],
                                xt[:, dc, tt * 128 : (tt + 1) * 128],
                                wv[:, dc, :],
                                start=(dc == 0),
                                stop=(dc == DC - 1),
                            )
                        nc.vector.tensor_copy(
                            vaug[:, gt, :, 0:DH],
                            ps[:, : HPC * DH].rearrange("p (h d) -> p h d", h=HPC),
                        )
                    return run

                pieces = []
                pi = 0
                for w_sb, dest in ((wq, qt), (wk, ktt)):
                    for p in range(2):
                        pieces.append(qk_piece(w_sb, dest, p, pi))
                        pi += 1
                for tt in range(4):
                    pieces.append(v_piece(tt))
                if defer_dma:
                    pieces.insert(0, issue_xt)
                return pieces

            # ---------------- out-projection pieces ----------------
            def gen_outproj_pieces(qc):
                def piece(tt):
                    def run():
                        gtt = qc * 4 + tt
                        ysb = ysbp.tile([128, 2 * TCH], F32, tag="ysb", name=f"ys{gtt}")
                        for ni in range(2):
                            ypsum = pjps.tile(
                                [128, TCH], F32, tag="pj", name=f"y{gtt}_{ni}"
                            )
                            for p2 in range(2):
                                nc.tensor.matmul(
                                    ypsum[:],
                                    ot[p2][:, gtt * 128 : (gtt + 1) * 128],
                                    wo[:, p2, ni * TCH : (ni + 1) * TCH],
                                    start=(p2 == 0),
                                    stop=(p2 == 1),
                                )
                            nc.vector.tensor_copy(
                                ysb[:, ni * TCH : (ni + 1) * TCH], ypsum[:]
                            )
                        nc.sync.dma_start(
                            y_d.ap()[gtt * 128 : (gtt + 1) * 128, :], ysb[:]
                        )
                    return run
                return [piece(tt) for tt in range(4)]

            # ---------------- flat attention pipeline ----------------
            # state shared across units
            pts = {}      # (qc,h) -> {kb: pt tile}
            opsums = {}   # (qc,h) -> psum tile
            oraws = {}    # (qc,h) -> oraw tile
            denps = {}    # (qc,hp) -> [2,TCH] den tile (partition-0 based)

            def emit_s_group(qc, h, g):
                """S matmuls + exp + mask for kb-pairs 2g, 2g+1 of unit."""
                nkt = 4 * qc + 4
                hp, par = divmod(h, 2)
                qsl = slice(qc * TCH, (qc + 1) * TCH)
                qrh = qt[hp][par * 64 : par * 64 + 64, qsl]
                for pairidx in (2 * g, 2 * g + 1):
                    kb = 2 * pairidx
                    if kb >= nkt:
                        continue
                    spt = sps.tile([128, 2, TCH], F32, tag="s", name=f"s{qc}_{h}_{kb}")
                    pt = ptp.tile([128, 2, TCH], BF16, tag="pt", name=f"pt{qc}_{h}_{kb}")
                    pts[(qc, h)][kb] = pt
                    offs = [max(0, 128 * (kb + j) - TCH * qc) for j in (0, 1)]
                    for j in (0, 1):
                        nc.tensor.matmul(
                            spt[:, j, offs[j] :],
                            ktt[hp][
                                par * 64 : par * 64 + 64,
                                (kb + j) * 128 : (kb + j + 1) * 128,
                            ],
                            qrh[:, offs[j] :],
                            start=True,
                            stop=True,
                        )
                    if offs[1] > 0:
                        for j in (0, 1):
                            nc.scalar.activation(
                                pt[:, j, offs[j] :],
                                spt[:, j, offs[j] :],
                                EXP,
                                bias=0.0,
                                scale=0.125,
                            )
                    else:
                        nc.scalar.activation(
                            pt.rearrange("p a b -> p (a b)"),
                            spt.rearrange("p a b -> p (a b)"),
                            EXP,
                            bias=0.0,
                            scale=0.125,
                        )
                    for j in (0, 1):
                        kt = kb + j
                        if kt >= 4 * qc:
                            off = offs[j]
                            if h % 2 == 1:
                                # causal mask as a DVE multiply with the
                                # constant triangular tile (gpsimd relief)
                                nc.vector.tensor_mul(
                                    pt[:, j, off:],
                                    pt[:, j, off:],
                                    maskt[:, : TCH - off],
                                )
                            else:
                                nc.gpsimd.affine_select(
                                    out=pt[:, j, off:],
                                    in_=pt[:, j, off:],
                                    compare_op=mybir.AluOpType.is_ge,
                                    fill=0.0,
                                    base=0,
                                    pattern=[[1, TCH - off]],
                                    channel_multiplier=-1,
                                )

            def emit_av_group(qc, h, g):
                nkt = 4 * qc + 4
                for pairidx in (2 * g, 2 * g + 1):
                    kb = 2 * pairidx
                    if kb >= nkt:
                        continue
                    if kb == 0:
                        opsums[(qc, h)] = ops.tile(
                            [128, TCH], F32, tag="o", name=f"o{qc}_{h}"
                        )
                    pt = pts[(qc, h)][kb]
                    for j in (0, 1):
                        kt = kb + j
                        off = max(0, 128 * kt - TCH * qc)
                        nc.tensor.matmul(
                            opsums[(qc, h)][0 : DH + 1, off:],
                            vaug[:, kt, h, :],
                            pt[:, j, off:],
                            start=(kt == 0),
                            stop=(kt == nkt - 1),
                        )

            def emit_evac(qc, h):
                hp, par = divmod(h, 2)
                if (qc, hp) not in denps:
                    denps[(qc, hp)] = miscp.tile(
                        [2, TCH], F32, tag=f"den{hp}", name=f"den{qc}_{hp}"
                    )
                oraw = orawp.tile([128, TCH], F32, tag="oraw", name=f"or{qc}_{h}")
                oraws[(qc, h)] = oraw
                nc.vector.tensor_copy(oraw[0 : DH + 1, :], opsums[(qc, h)][0 : DH + 1, :])
                nc.sync.dma_start(denps[(qc, hp)][par : par + 1, :], oraw[DH : DH + 1, :])

            def emit_norm_pair(qc, hp):
                """Fast approx reciprocal, matmul-broadcast to 128 partitions,
                then scale into ot. ot layout is par-swapped: partitions 0-63
                hold the odd head of the pair (DVE-direct write), 64-127 the
                even head (via DMA). Emitted per pair, as soon as that pair's
                AV is done, so only the last pair sits on the kernel tail."""
                qsl = slice(qc * TCH, (qc + 1) * TCH)
                denr = miscp.tile([2, TCH], F32, tag=f"denr{hp}", name=f"dr{qc}_{hp}")
                nc.vector.reciprocal_approx_fast(out=denr[:], in_=denps[(qc, hp)][:])
                bcps = ops.tile([128, TCH], F32, tag="o", name=f"bc{qc}_{hp}")
                nc.tensor.matmul(bcps[:], sel[hp][:], denr[:], start=True, stop=True)
                h_hi, h_lo = 2 * hp + 1, 2 * hp
                nc.vector.tensor_mul(
                    ot[hp][0:64, qsl], oraws[(qc, h_hi)][0:64, :], bcps[0:64, :]
                )
                tmpo = miscp.tile([64, TCH], BF16, tag="tmpo", name=f"tp{qc}_{hp}")
                nc.vector.tensor_mul(
                    tmpo[:], oraws[(qc, h_lo)][0:64, :], bcps[64:128, :]
                )
                nc.sync.dma_start(ot[hp][64:128, qsl], tmpo[:])

            # ---------------- emission schedule ----------------
            p0 = gen_proj_pieces(0)
            p1 = gen_proj_pieces(1, defer_dma=True)
            # stagger the remaining preloads between proj(0) pieces so the
            # startup DMA bandwidth goes to wq+xt0 first
            order01 = (
                [p0[0], p0[1], p0[2], load_wv, p1[0]]
                + p0[3:]
                + p1[1:]
            )
            for f in order01:
                f()

            units = [(qc, h) for qc in CHUNK_ORDER for h in range(HPC)]
            # filler pieces attached per unit (consumed one per S-group slot)
            unit_fillers = {i: [] for i in range(len(units))}

            def attach(fillers, lo, hi):
                n = hi - lo
                for i, f in enumerate(fillers):
                    unit_fillers[lo + i * n // len(fillers)].append(f)

            attach(gen_proj_pieces(2, defer_dma=True), 0, 4)  # during chunk 1

            def load_wo():
                nc.gpsimd.dma_start(
                    wo[:], wo_d.ap().rearrange("(c p) n -> p c n", p=128)
                )

            # keys must come AFTER the unit that emits the norm the pieces read
            # (chunk order 1,2,3,0: norm(1)@ui4, norm(2)@ui8, norm(3)@ui12)
            lazy_fillers = {
                0: lambda: [load_wo, load_sel],
                4: lambda: gen_proj_pieces(3, defer_dma=True),  # chunk 2 units
                8: lambda: gen_outproj_pieces(1),   # chunk 3 units
                12: lambda: gen_outproj_pieces(2),  # chunk 0 units (they are
                13: lambda: gen_outproj_pieces(3),  # latency-bound, need fill)
            }

            for ui, (qc, h) in enumerate(units):
                if ui in lazy_fillers:
                    pieces = lazy_fillers[ui]()
                    hi = min(ui + 4, len(units))
                    attach(pieces, ui, hi)
                pts[(qc, h)] = {}
                nkt = 4 * qc + 4
                ngroups = (nkt // 2 + 1) // 2  # S groups of 2 pairs
                prev = units[ui - 1] if ui > 0 else None
                pgroups = ((4 * prev[0] + 4) // 2 + 1) // 2 if prev else 0
                fill = unit_fillers[ui]
                for g in range(max(ngroups, pgroups)):
                    if fill:
                        fill.pop(0)()
                    if g < ngroups:
                        emit_s_group(qc, h, g)
                    if prev and g < pgroups:
                        emit_av_group(prev[0], prev[1], g)
                for f in fill:
                    f()
                fill.clear()
                if prev:
                    emit_evac(prev[0], prev[1])
                    if prev[1] == 1:
                        emit_norm_pair(prev[0], 0)
                    elif prev[1] == 3:
                        emit_norm_pair(prev[0], 1)

            # drain: AV + norm of the final unit, then outproj of last chunk
            qc_l, h_l = units[-1]
            for g in range(((4 * qc_l + 4) // 2 + 1) // 2):
                emit_av_group(qc_l, h_l, g)
            emit_evac(qc_l, h_l)
            emit_norm_pair(qc_l, 1)
            for f in gen_outproj_pieces(qc_l):
                f()
    nc.compile()
    return nc


_NC_CACHE = []


def _get_nc():
    if not _NC_CACHE:
        _NC_CACHE.append(_build())
    return _NC_CACHE[0]


_LAST_RESULTS = []  # stashed BassKernelResults for test harness introspection


def _wo_rows_parswap(Wout_rows):
    """Reorder the 256 Wout rows so each 128-row pair block is [odd-head 64 | even-head 64]."""
    out = np.empty_like(Wout_rows)
    for hp in range(2):
        blk = Wout_rows[hp * 128 : (hp + 1) * 128]
        out[hp * 128 : hp * 128 + 64] = blk[64:128]
        out[hp * 128 + 64 : (hp + 1) * 128] = blk[0:64]
    return out


def kernel(x, Wqkv, Wout, _trace=False, **_trace_kwargs):
    x = np.asarray(x, dtype=np.float32)
    Wqkv = np.asarray(Wqkv, dtype=np.float32)
    Wout = np.asarray(Wout, dtype=np.float32)

    nc = _get_nc()
    in_maps = []
    for c in range(NCORES):
        b, g = divmod(c, HPC)
        cols = slice(g * HPC * DH, (g + 1) * HPC * DH)
        rows = slice(g * HPC * DH, (g + 1) * HPC * DH)
        bf = ml_dtypes.bfloat16
        in_maps.append(
            {
                "xT": np.ascontiguousarray(x[b].T.astype(bf)),
                "wq": np.ascontiguousarray(Wqkv[:, 0:D][:, cols].astype(bf)),
                "wk": np.ascontiguousarray(Wqkv[:, D : 2 * D][:, cols].astype(bf)),
                "wv": np.ascontiguousarray(Wqkv[:, 2 * D : 3 * D][:, cols].astype(bf)),
                "wo": np.ascontiguousarray(
                    _wo_rows_parswap(Wout[rows, :]).astype(bf)
                ),
            }
        )

    res = run_bass_kernel_spmd(
        nc, in_maps, core_ids=list(range(NCORES)), trace=_trace, **_trace_kwargs
    )
    _LAST_RESULTS.clear()
    _LAST_RESULTS.append(res)

    out = np.zeros((B, T, D), dtype=np.float32)
    for c in range(NCORES):
        b = c // HPC
        out[b] += res.results[c]["y"]
    return out
